# revision 58
# baseline (speedup 1.0000x reference)
"""BEiT self-attention Trainium2 kernel (Bass/Tile), data-parallel over batch on 8 cores.

bf16 layout strategy (per core, 8 batches):
  - hidden pre-transposed on host to feature-major xT [768, 1584] bf16 (padded).
  - Q^T, K^T computed head-dim-major [o, m] in bf16; 1/sqrt(64) folded into wq/bq.
    Q gets bias+cast on ACT, K casts on ACT.
  - V computed seq-major per (batch, j-tile) with a ones column per head
    (65-wide head groups) so the probs@V matmul also yields softmax row-sums.
  - Attention per (batch, head): scoresT[j, i] = k^T.T @ q^T streamed over a
    198-wide i window, two heads paired on opposite PE row halves.
    Softmax bias is folded multiplicatively: u = exp(scores) * expb where
    expb = exp(rel_bias) is precomputed on host (bf16).
  - ctx computed directly seq-major: ct[i, 65] = u[j,i].T @ [V|1][j, 65]
    (u tiles as PE weights) - no PE transpose, no PSUM->SBUF copy.
    ct accumulates 6 heads side by side [i, 390]; normalization is one
    reciprocal + one broadcast-multiply per (batch, i-tile, head-group).
  - Work split into 4 groups of 2 batches; group g's attention interleaves
    with group g+1's projection matmuls to keep the PE stream dense.
"""

from collections import deque

import numpy as np
import ml_dtypes

import concourse.bacc as bacc
import concourse.mybir as mybir
from concourse.tile import TileContext
from concourse.bass import broadcast_tensor_aps as bass_broadcast
from concourse.bass_utils import run_bass_kernel_spmd

B, S, D, H, HD = 64, 197, 768, 12, 64
NCORES = 8
BPC = B // NCORES  # batches per core
F32 = mybir.dt.float32
BF16 = mybir.dt.bfloat16
IW = 198  # i window per j-tile half
IW2 = 2 * IW  # 396
KT = D // 128  # 6 contraction tiles
OT = D // 128  # 6 output-feature tiles
JT = [(0, 128), (128, S - 128)]  # j (key) partition tiles
IT = [(0, 128), (128, S - 128)]  # i (query) partition tiles
MG = 2 * S  # group width (2 batches)
QW = MG + 2  # padded stream width (scores windows reach col 395)
XW = BPC * S + 32  # padded xT dram width (xbar V transposes read j-tiles padded to 32)
AluOp = mybir.AluOpType
ActFn = mybir.ActivationFunctionType


def _round_up(size):
    for v in (32, 64, 128):
        if v >= size:
            return v
    raise AssertionError(size)


def matmul_noldw(te, out, lhsT, rhs, start, stop):
    """InstMatmult with ldweights=False: uses the weights already resident in
    the PE array (loaded by a preceding explicit nc.tensor.ldweights)."""
    keep_dims = {0}
    ifmap_ap = te.lower_ap(rhs.opt(keep_dims), opt=False)
    weights_ap = te.lower_ap(lhsT.opt(keep_dims), opt=False, for_matmul_weights=True)
    out_ap = te.lower_ap(out)
    tile_size = (_round_up(rhs.partition_size()), _round_up(out.partition_size()))
    tile_position = (lhsT.base_partition(), out.base_partition())
    return te.add_instruction(
        mybir.InstMatmult(
            name=te.bass.get_next_instruction_name(),
            replication_resolution=0,
            replication_shift_amnt=0,
            replication_num_rows=0,
            start_tensor_calc=start,
            stop_tensor_calc=stop,
            ins=[ifmap_ap, weights_ap],
            outs=[out_ap],
            perf_mode=None,
            is_transpose=None,
            ifmap_quant_offset=None,
            weights_quant_offset=None,
            bass_skip_group_check=False,
            tile_position=tile_position,
            tile_size=tile_size,
            ldweights=False,
        )
    )


def build_program(
    bpc=BPC,
    group_sizes=None,
    startup_split=False,
    ubufs=9,
    ebufs=8,
    wq_half=False,
    umult_gpsimd=False,
    kt_dve=False,
    v_kiouter=False,
    warmup=0,
    lag=4,
    last_lag=6,
    v_xbar=False,
    sc_ldw1=False,
    v_ldw1=False,
    out_bf16=True,
    proj_first=False,
    ctbufs=4,
):
    if group_sizes is None:
        group_sizes = (2, 2, 2, 2) if bpc == 8 else (bpc,)
    assert sum(group_sizes) == bpc
    MTOT = bpc * S

    nc = bacc.Bacc("TRN2", target_bir_lowering=False, debug=False, num_devices=1)
    xT_d = nc.dram_tensor("xT", [D, XW], BF16, kind="ExternalInput")
    wqT_d = nc.dram_tensor("wqT", [D, D], BF16, kind="ExternalInput")
    wkT_d = nc.dram_tensor("wkT", [D, D], BF16, kind="ExternalInput")
    wvT_d = nc.dram_tensor("wvT", [D, D], BF16, kind="ExternalInput")
    bq_d = nc.dram_tensor("bq2", [128, 2 * OT], F32, kind="ExternalInput")
    bv_d = nc.dram_tensor("bvb", [128, D], BF16, kind="ExternalInput")
    eb_d = nc.dram_tensor("expb", [128, H * IW2], BF16, kind="ExternalInput")
    on_d = nc.dram_tensor("onec", [128, H], BF16, kind="ExternalInput")
    out_d = nc.dram_tensor(
        "out", [MTOT, D], BF16 if out_bf16 else F32, kind="ExternalOutput"
    )

    with TileContext(nc) as tc:
        with (
            tc.tile_pool(name="const", bufs=1) as cp,
            tc.tile_pool(name="grp", bufs=2) as gp,
            tc.tile_pool(name="work", bufs=3) as wp,
            tc.tile_pool(name="ps", bufs=1, space="PSUM") as pp,
        ):
            wq_t = [
                cp.tile([128, D], BF16, name=f"wq{k}", tag=f"wq{k}") for k in range(KT)
            ]
            wk_t = [
                cp.tile([128, D], BF16, name=f"wk{k}", tag=f"wk{k}") for k in range(KT)
            ]
            wv_t = [
                cp.tile([128, D], BF16, name=f"wv{k}", tag=f"wv{k}") for k in range(KT)
            ]
            bqs = cp.tile([128, 2 * OT], F32, tag="bqs")
            bvb = cp.tile([128, D], BF16, tag="bvb")
            onec = cp.tile([128, H], BF16, tag="onec")
            ebf = cp.tile([128, H * IW2], BF16, tag="ebf")
            eb_t = [ebf[:, h * IW2 : (h + 1) * IW2] for h in range(H)]
            xf = [
                cp.tile([128, XW], BF16, name=f"xf{k}", tag=f"xf{k}") for k in range(KT)
            ]

            def load_small():
                nc.sync.dma_start(bqs[:], bq_d[:, :])
                nc.sync.dma_start(bvb[:], bv_d[:, :])
                nc.sync.dma_start(onec[:], on_d[:, :])

            def load_wq_x0():
                # wq + group-0's x slice interleaved: the first qt chain only
                # depends on these; o=0 weight columns land first so qt(0)
                # can start as soon as ~130KB are in.
                if startup_split:
                    for k in range(KT):
                        nc.sync.dma_start(
                            wq_t[k][:, :128], wqT_d[k * 128 : (k + 1) * 128, :128]
                        )
                        nc.sync.dma_start(
                            xf[k][:, :QW], xT_d[k * 128 : (k + 1) * 128, :QW]
                        )
                    for k in range(KT):
                        nc.sync.dma_start(
                            wq_t[k][:, 128:], wqT_d[k * 128 : (k + 1) * 128, 128:]
                        )
                elif wq_half:
                    for k in range(KT):
                        nc.sync.dma_start(
                            wq_t[k][:, :384], wqT_d[k * 128 : (k + 1) * 128, :384]
                        )
                        nc.sync.dma_start(
                            xf[k][:, :QW], xT_d[k * 128 : (k + 1) * 128, :QW]
                        )
                    for k in range(KT):
                        nc.sync.dma_start(
                            wq_t[k][:, 384:], wqT_d[k * 128 : (k + 1) * 128, 384:]
                        )
                else:
                    for k in range(KT):
                        nc.sync.dma_start(wq_t[k][:], wqT_d[k * 128 : (k + 1) * 128, :])
                        nc.sync.dma_start(
                            xf[k][:, :QW], xT_d[k * 128 : (k + 1) * 128, :QW]
                        )

            def load_wk_xrest():
                # wk first: the kt chains stall on it; the bulk x transfer is
                # only needed by group 1's projections much later
                for k in range(KT):
                    nc.sync.dma_start(wk_t[k][:], wkT_d[k * 128 : (k + 1) * 128, :])
                for k in range(KT):
                    nc.sync.dma_start(
                        xf[k][:, QW:], xT_d[k * 128 : (k + 1) * 128, QW:]
                    )

            def load_wv():
                for k in range(KT):
                    nc.sync.dma_start(wv_t[k][:], wvT_d[k * 128 : (k + 1) * 128, :])

            def load_bias():
                nc.sync.dma_start(ebf[:], eb_d[:, :])

            def proj_pieces(g, GB, b0):
                """Emission thunks for group g's projections; last item is the
                ('ctx', dict) sentinel carrying the produced tiles."""
                m0 = b0 * S
                gw = GB * S + 2  # used stream width (<= QW tile width)
                ctx = {}

                def piece_load():
                    ctx["xt"] = [xf[k][:, m0 : m0 + gw] for k in range(KT)]
                    ctx["qt"] = [
                        gp.tile([128, QW], BF16, name=f"qt{o}", tag=f"qt{o}")
                        for o in range(OT)
                    ]
                    ctx["kt"] = [
                        gp.tile([128, QW], BF16, name=f"kt{o}", tag=f"kt{o}")
                        for o in range(OT)
                    ]
                    ctx["vt"] = {}

                yield piece_load

                def piece_qt(o):
                    xt, qt = ctx["xt"], ctx["qt"]
                    ps = pp.tile([128, 512], F32, name="pp", tag="mm512", bufs=4)
                    for ki in range(KT):
                        nc.tensor.matmul(
                            ps[:, :gw],
                            wq_t[ki][:, o * 128 : (o + 1) * 128],
                            xt[ki][:, :gw],
                            start=(ki == 0),
                            stop=(ki == KT - 1),
                        )
                    nc.scalar.activation(
                        qt[o][:, :gw],
                        ps[:, :gw],
                        ActFn.Identity,
                        bias=bqs[:, o : o + 1],
                    )

                def piece_kt(o):
                    xt, kt = ctx["xt"], ctx["kt"]
                    ps = pp.tile([128, 512], F32, name="pp", tag="mm512", bufs=4)
                    for ki in range(KT):
                        nc.tensor.matmul(
                            ps[:, :gw],
                            wk_t[ki][:, o * 128 : (o + 1) * 128],
                            xt[ki][:, :gw],
                            start=(ki == 0),
                            stop=(ki == KT - 1),
                        )
                    if kt_dve:
                        nc.vector.tensor_copy(kt[o][:, :gw], ps[:, :gw])
                    else:
                        nc.scalar.activation(kt[o][:, :gw], ps[:, :gw], ActFn.Identity)

                def piece_v(b, jt):
                    xt = ctx["xt"]
                    j0, jw = JT[jt]
                    v = gp.tile(
                        [128, H * 65], BF16, name=f"v{b}_{jt}", tag=f"v{b}_{jt}"
                    )
                    v3 = v[:jw, :].rearrange("p (h c) -> p h c", c=65)
                    chunks = [(0, 512, 0), (512, 256, 8)]
                    pss = [
                        pp.tile([128, 512], F32, name="pp", tag="mm512", bufs=4)
                        for _ in chunks
                    ]
                    if v_ldw1:
                        # ki outer with one explicit weight load per ki shared
                        # by both chunk matmuls
                        for ki in range(KT):
                            wap = xt[ki][:, b * S + j0 : b * S + j0 + jw]
                            nc.tensor.ldweights(wap)
                            for ps, (c0, cw, h0) in zip(pss, chunks):
                                matmul_noldw(
                                    nc.tensor,
                                    ps[:jw, :cw],
                                    wap,
                                    wv_t[ki][:, c0 : c0 + cw],
                                    start=(ki == 0),
                                    stop=(ki == KT - 1),
                                )
                    elif v_kiouter:
                        # ki outer: the two chunk matmuls per ki share the same
                        # stationary operand (the xt j-slice)
                        for ki in range(KT):
                            for ps, (c0, cw, h0) in zip(pss, chunks):
                                nc.tensor.matmul(
                                    ps[:jw, :cw],
                                    xt[ki][:, b * S + j0 : b * S + j0 + jw],
                                    wv_t[ki][:, c0 : c0 + cw],
                                    start=(ki == 0),
                                    stop=(ki == KT - 1),
                                )
                    else:
                        for ps, (c0, cw, h0) in zip(pss, chunks):
                            for ki in range(KT):
                                nc.tensor.matmul(
                                    ps[:jw, :cw],
                                    xt[ki][:, b * S + j0 : b * S + j0 + jw],
                                    wv_t[ki][:, c0 : c0 + cw],
                                    start=(ki == 0),
                                    stop=(ki == KT - 1),
                                )
                    for ps, (c0, cw, h0) in zip(pss, chunks):
                        nh = cw // 64
                        dst = v3[:, h0 : h0 + nh, 0:64]
                        src = ps[:jw, :cw].rearrange("p (h c) -> p h c", c=64)
                        bsl = bvb[:jw, c0 : c0 + cw].rearrange("p (h c) -> p h c", c=64)
                        nc.vector.tensor_tensor(dst, src, bsl, AluOp.add)
                    nc.vector.tensor_copy(
                        v3[:, :, 64:65],
                        onec[:jw, :].rearrange("p (h c) -> p h c", c=1),
                    )
                    ctx["vt"][b, jt] = v

                def piece_vT(o):
                    xt = ctx["xt"]
                    ps = pp.tile([128, 512], F32, name="pp", tag="mm512", bufs=4)
                    for ki in range(KT):
                        nc.tensor.matmul(
                            ps[:, :gw],
                            wv_t[ki][:, o * 128 : (o + 1) * 128],
                            xt[ki][:, :gw],
                            start=(ki == 0),
                            stop=(ki == KT - 1),
                        )
                    vT = gp.tile([128, QW + 64], BF16, name=f"vT{o}", tag=f"vT{o}")
                    nc.scalar.activation(
                        vT[:, :gw],
                        ps[:, :gw],
                        ActFn.Identity,
                        bias=bqs[:, OT + o : OT + o + 1],
                    )
                    ctx.setdefault("vT", {})[o] = vT

                def piece_vx(b, jt):
                    j0, jw = JT[jt]
                    jw2 = 128  # xbar transpose needs 128-divisible source cols
                    v = gp.tile(
                        [128, H * 65], BF16, name=f"v{b}_{jt}", tag=f"v{b}_{jt}"
                    )
                    v3p = v[:jw2, :].rearrange("p (h c) -> p h c", c=65)
                    for o in range(OT):
                        nc.sync.dma_start_transpose(
                            v3p[:, 2 * o : 2 * o + 2, 0:64],
                            ctx["vT"][o][:, b * S + j0 : b * S + j0 + jw2],
                        )
                    nc.vector.tensor_copy(
                        v[:jw, :].rearrange("p (h c) -> p h c", c=65)[:, :, 64:65],
                        onec[:jw, :].rearrange("p (h c) -> p h c", c=1),
                    )
                    ctx["vt"][b, jt] = v

                for o in range(OT):
                    yield (lambda o=o: piece_qt(o))
                for o in range(OT):
                    yield (lambda o=o: piece_kt(o))
                if v_xbar:
                    for o in range(OT):
                        yield (lambda o=o: piece_vT(o))
                    for b in range(GB):
                        for jt in range(2):
                            yield (lambda b=b, jt=jt: piece_vx(b, jt))
                else:
                    for b in range(GB):
                        for jt in range(2):
                            yield (lambda b=b, jt=jt: piece_v(b, jt))
                yield ("ctx", ctx)

            def att_pieces(GB, b0, ctx, lag=lag):
                """Emission thunks for a group's attention (lagged ctx stage).
                ctx is read lazily: tiles may be created mid-stream."""
                hgstate = {}

                def stage_scores_pair(b, hp):
                    qt, kt = ctx["qt"], ctx["kt"]
                    o = hp // 2
                    sts = [
                        pp.tile([128, 512], F32, name="st", tag="mm512", bufs=4)
                        for _ in range(2)
                    ]
                    # interleave the two heads so consecutive matmuls hit
                    # opposite PE row groups (partitions 0-63 vs 64-127) and
                    # overlap in the array
                    for jt, (j0, jw) in enumerate(JT):
                        if sc_ldw1:
                            # one full-array load covers both heads' row halves
                            nc.tensor.ldweights(
                                kt[o][0:128, b * S + j0 : b * S + j0 + jw]
                            )
                        for dh in (0, 1):
                            po = dh * 64
                            if sc_ldw1:
                                matmul_noldw(
                                    nc.tensor,
                                    sts[dh][:jw, jt * IW : (jt + 1) * IW],
                                    kt[o][po : po + 64, b * S + j0 : b * S + j0 + jw],
                                    qt[o][po : po + 64, b * S : b * S + IW],
                                    start=True,
                                    stop=True,
                                )
                            else:
                                nc.tensor.matmul(
                                    sts[dh][:jw, jt * IW : (jt + 1) * IW],
                                    kt[o][po : po + 64, b * S + j0 : b * S + j0 + jw],
                                    qt[o][po : po + 64, b * S : b * S + IW],
                                    start=True,
                                    stop=True,
                                )
                    out = []
                    for dh in (0, 1):
                        e = wp.tile([128, IW2], BF16, name="ee", tag="ee", bufs=ebufs)
                        nc.scalar.activation(e[:, :], sts[dh][:, :IW2], ActFn.Exp)
                        u = wp.tile([128, IW2], BF16, name="uu", tag="uu", bufs=ubufs)
                        ueng = nc.gpsimd if umult_gpsimd else nc.vector
                        ueng.tensor_tensor(
                            u[:, :], e[:, :], eb_t[hp + dh][:, :], AluOp.mult
                        )
                        out.append(u)
                    return out

                def stage_ctx(b, h, u, row0):
                    vt = ctx["vt"]
                    hg, hl = h // 6, h % 6
                    if hl == 0:
                        hgstate[b, hg] = [
                            pp.tile([128, 390], F32, name="ct", tag="ct", bufs=ctbufs)
                            for _ in range(2)
                        ]
                    cts = hgstate[b, hg]
                    for it, (i0, iw) in enumerate(IT):
                        for jt, (j0, jw) in enumerate(JT):
                            nc.tensor.matmul(
                                cts[it][:iw, hl * 65 : (hl + 1) * 65],
                                u[:jw, jt * IW + i0 : jt * IW + i0 + iw],
                                vt[b, jt][:jw, h * 65 : (h + 1) * 65],
                                start=(jt == 0),
                                stop=(jt == 1),
                            )
                    if hl == 5:
                        for it, (i0, iw) in enumerate(IT):
                            ct3 = cts[it][:iw, :].rearrange("p (h c) -> p h c", c=65)
                            rt = wp.tile([128, 6], F32, name="rt", tag="rt")
                            rt3 = rt[:iw, :].rearrange("p (h c) -> p h c", c=1)
                            nc.vector.reciprocal(rt3, ct3[:, :, 64:65])
                            num = ct3[:, :, 0:64]
                            _, rb3 = bass_broadcast(num, rt3)
                            ob = wp.tile(
                                [128, 384], BF16 if out_bf16 else F32, name="ob", tag="ob"
                            )
                            nc.vector.tensor_tensor(
                                ob[:iw, :].rearrange("p (h c) -> p h c", c=64),
                                num,
                                rb3,
                                AluOp.mult,
                            )
                            nc.sync.dma_start(
                                out_d[
                                    row0 + i0 : row0 + i0 + iw,
                                    hg * 384 : (hg + 1) * 384,
                                ],
                                ob[:iw, :],
                            )

                pend = deque()
                for b in range(GB):
                    for hp in range(0, H, 2):

                        def piece(b=b, hp=hp):
                            us01 = stage_scores_pair(b, hp)
                            for dh in (0, 1):
                                pend.append((b, hp + dh, us01[dh], (b0 + b) * S))
                            while len(pend) > lag:
                                stage_ctx(*pend.popleft())

                        yield piece

                def flush():
                    while pend:
                        stage_ctx(*pend.popleft())

                yield flush

            def run_proj(gen):
                pieces = []
                ctx = None
                for item in gen:
                    if isinstance(item, tuple) and item[0] == "ctx":
                        ctx = item[1]
                    else:
                        pieces.append(item)
                return pieces, ctx

            b0s = []
            acc = 0
            for GB in group_sizes:
                b0s.append(acc)
                acc += GB

            g0_pieces, prev_ctx = run_proj(proj_pieces(0, group_sizes[0], b0s[0]))
            load_small()
            if warmup:
                # dummy matmuls on a tiny early tile: keep the PE busy during
                # the input DMA lead-in so HAM ramps the clock before real work
                dum = cp.tile([128, 512], BF16, tag="dum")
                nc.sync.dma_start(dum[:], wqT_d[0:128, 0:512])
                for w in range(warmup):
                    psw = pp.tile([128, 512], F32, name="pp", tag="mm512", bufs=4)
                    nc.tensor.matmul(
                        psw[:, :512],
                        dum[:, 0:128],
                        dum[:, :512],
                        start=True,
                        stop=True,
                    )
            load_wq_x0()
            g0_pieces[0]()  # tile setup only
            pq0 = g0_pieces[1 : 1 + OT]
            pk0 = g0_pieces[1 + OT : 1 + 2 * OT]
            pv0 = g0_pieces[1 + 2 * OT :]
            pq0[0]()
            load_wk_xrest()
            for p in pq0[1:]:
                p()
            load_wv()
            for p in pk0:
                p()
            load_bias()
            for p in pv0:
                p()

            def interleave(astream, pstream):
                ratio = max(1, len(astream) // max(1, len(pstream)))
                out = []
                ai = pi = 0
                while ai < len(astream) or pi < len(pstream):
                    if proj_first and pi < len(pstream):
                        out.append(pstream[pi])
                        pi += 1
                    for _ in range(ratio):
                        if ai < len(astream):
                            out.append(astream[ai])
                            ai += 1
                    if not proj_first and pi < len(pstream):
                        out.append(pstream[pi])
                        pi += 1
                return out

            ng = len(group_sizes)
            for g in range(1, ng - 1):
                pieces, g_ctx = run_proj(proj_pieces(g, group_sizes[g], b0s[g]))
                for p in interleave(
                    list(att_pieces(group_sizes[g - 1], b0s[g - 1], prev_ctx)), pieces
                ):
                    p()
                prev_ctx = g_ctx

            if ng == 1:
                for p in att_pieces(group_sizes[0], b0s[0], prev_ctx):
                    p()
            else:
                # final window: att(gl-1) interleaved with the last group's
                # load/qt/kt0-2 pieces; kt3-5 + V pieces are deferred into the
                # last group's own attention stream as just-in-time PE filler.
                gl = ng - 1
                pieces, gl_ctx = run_proj(proj_pieces(gl, group_sizes[gl], b0s[gl]))
                pload = pieces[0]
                pqt = pieces[1 : 1 + OT]
                pkt = pieces[1 + OT : 1 + 2 * OT]
                window = [pload] + pqt + pkt[:3]
                for p in interleave(
                    list(att_pieces(group_sizes[gl - 1], b0s[gl - 1], prev_ctx)),
                    window,
                ):
                    p()
                if v_xbar:
                    pvT = pieces[1 + 2 * OT : 1 + 3 * OT]
                    pvx = pieces[1 + 3 * OT :]
                    apieces = list(
                        att_pieces(group_sizes[gl], b0s[gl], gl_ctx, lag=8)
                    )
                    aflush = apieces[-1]
                    A = apieces[:-1]
                    sched = {
                        0: [pvT[0], pvT[1]],
                        1: [pvT[2], pvT[3]],
                        2: [pvT[4], pvT[5]],
                        3: [pkt[3]] + pvx[0:2],
                        4: [pkt[4]],
                        5: [pkt[5]],
                        6: pvx[2:],
                    }
                    out_stream = []
                    for idx, a in enumerate(A):
                        out_stream.extend(sched.get(idx, []))
                        out_stream.append(a)
                    out_stream.append(aflush)
                    for p in out_stream:
                        p()
                else:
                    pv = deque(pieces[1 + 2 * OT :])
                    apieces = list(
                        att_pieces(group_sizes[gl], b0s[gl], gl_ctx, lag=last_lag)
                    )
                    aflush = apieces[-1]
                    A = apieces[:-1]
                    out_stream = []
                    if last_lag >= 8:
                        pv_sched = {0: 2, 8: 1, 9: 1}
                    elif last_lag >= 6:
                        # deeper lag delays the second batch's ctx pops, so its
                        # V pieces can fill the otherwise-empty late pairs
                        pv_sched = {0: 2, 7: 1, 8: 1}
                    else:
                        pv_sched = {0: 2, 5: 1, 6: 1}
                    for idx, a in enumerate(A):
                        if 3 <= idx < OT:
                            out_stream.append(pkt[idx])  # kt[idx] just before its pair
                        out_stream.append(a)
                        for _ in range(pv_sched.get(idx, 0)):
                            if pv:
                                out_stream.append(pv.popleft())
                    while pv:
                        out_stream.append(pv.popleft())
                    out_stream.append(aflush)
                    for p in out_stream:
                        p()

    nc.compile()
    return nc


def prep_host_inputs(inputs, bpc=BPC, cores=NCORES):
    """Shared (per-core-identical) tensors + per-core xT shards."""
    hs = np.ascontiguousarray(np.asarray(inputs["hidden_states"], dtype=np.float32))
    wq = np.asarray(inputs["wq"], np.float32)
    bq = np.asarray(inputs["bq"], np.float32)
    wk = np.asarray(inputs["wk"], np.float32)
    wv = np.asarray(inputs["wv"], np.float32)
    bv = np.asarray(inputs["bv"], np.float32)
    bias_table = np.asarray(inputs["bias_table"], np.float32)
    rel_index = np.asarray(inputs["rel_index"])

    bf = ml_dtypes.bfloat16
    scale = np.float32(1.0 / np.sqrt(HD))
    common = {
        "wqT": np.ascontiguousarray(wq.T * scale).astype(bf),
        "wkT": np.ascontiguousarray(wk.T).astype(bf),
        "wvT": np.ascontiguousarray(wv.T).astype(bf),
        "bq2": np.ascontiguousarray(
            np.concatenate(
                [(bq * scale).reshape(OT, 128).T, bv.reshape(OT, 128).T], axis=1
            )
        ),
        "bvb": np.ascontiguousarray(np.broadcast_to(bv, (128, D))).astype(bf),
        "onec": np.ones((128, H), bf),
    }
    rb = bias_table[rel_index]  # [i, j, H]
    bjiT = rb.transpose(2, 1, 0)  # [h, j, i]
    eb = np.zeros((H, 128, IW2), np.float32)
    for jt, (j0, jw) in enumerate(JT):
        eb[:, :jw, jt * IW : jt * IW + S] = np.exp(bjiT[:, j0 : j0 + jw, :])
    # [h, p, c] -> [p, h*IW2 + c]
    common["expb"] = np.ascontiguousarray(eb.transpose(1, 0, 2).reshape(128, H * IW2)).astype(bf)

    in_maps = []
    for c in range(cores):
        xc = hs[c * bpc : (c + 1) * bpc].reshape(bpc * S, D)
        xT = np.zeros((D, XW), np.float32)
        xT[:, : bpc * S] = xc.T
        in_maps.append({"xT": xT.astype(bf), **common})
    return in_maps


_prog_cache = {}


def get_program(bpc=BPC, group_sizes=None, **kw):
    key = (bpc, group_sizes, tuple(sorted(kw.items())))
    if key not in _prog_cache:
        _prog_cache[key] = build_program(bpc, group_sizes, **kw)
    return _prog_cache[key]


def kernel(**inputs):
    nc = get_program()
    in_maps = prep_host_inputs(inputs)
    res = run_bass_kernel_spmd(nc, in_maps, list(range(NCORES)))
    out = np.concatenate(
        [np.asarray(res.results[c]["out"], dtype=np.float32) for c in range(NCORES)],
        axis=0,
    )
    return out.reshape(B, S, D)


# revision 62
# speedup vs baseline: 1.0066x; 1.0066x over previous
"""BEiT self-attention Trainium2 kernel (Bass/Tile), data-parallel over batch on 8 cores.

bf16 layout strategy (per core, 8 batches):
  - hidden pre-transposed on host to feature-major xT [768, 1584] bf16 (padded).
  - Q^T, K^T computed head-dim-major [o, m] in bf16; 1/sqrt(64) folded into wq/bq.
    Q gets bias+cast on ACT, K casts on ACT.
  - V computed seq-major per (batch, j-tile) with a ones column per head
    (65-wide head groups) so the probs@V matmul also yields softmax row-sums.
  - Attention per (batch, head): scoresT[j, i] = k^T.T @ q^T streamed over a
    198-wide i window, two heads paired on opposite PE row halves.
    Softmax bias is folded multiplicatively: u = exp(scores) * expb where
    expb = exp(rel_bias) is precomputed on host (bf16).
  - ctx computed directly seq-major: ct[i, 65] = u[j,i].T @ [V|1][j, 65]
    (u tiles as PE weights) - no PE transpose, no PSUM->SBUF copy.
    ct accumulates 6 heads side by side [i, 390]; normalization is one
    reciprocal + one broadcast-multiply per (batch, i-tile, head-group).
  - Work split into 4 groups of 2 batches; group g's attention interleaves
    with group g+1's projection matmuls to keep the PE stream dense.
"""

from collections import deque

import numpy as np
import ml_dtypes

import concourse.bacc as bacc
import concourse.mybir as mybir
from concourse.tile import TileContext
from concourse.bass import broadcast_tensor_aps as bass_broadcast
from concourse.bass_utils import run_bass_kernel_spmd

B, S, D, H, HD = 64, 197, 768, 12, 64
NCORES = 8
BPC = B // NCORES  # batches per core
F32 = mybir.dt.float32
BF16 = mybir.dt.bfloat16
IW = 198  # i window per j-tile half
IW2 = 2 * IW  # 396
KT = D // 128  # 6 contraction tiles
OT = D // 128  # 6 output-feature tiles
JT = [(0, 128), (128, S - 128)]  # j (key) partition tiles
IT = [(0, 128), (128, S - 128)]  # i (query) partition tiles
MG = 2 * S  # group width (2 batches)
QW = MG + 2  # padded stream width (scores windows reach col 395)
XW = BPC * S + 32  # padded xT dram width (xbar V transposes read j-tiles padded to 32)
AluOp = mybir.AluOpType
ActFn = mybir.ActivationFunctionType


def _round_up(size):
    for v in (32, 64, 128):
        if v >= size:
            return v
    raise AssertionError(size)


def matmul_noldw(te, out, lhsT, rhs, start, stop):
    """InstMatmult with ldweights=False: uses the weights already resident in
    the PE array (loaded by a preceding explicit nc.tensor.ldweights)."""
    keep_dims = {0}
    ifmap_ap = te.lower_ap(rhs.opt(keep_dims), opt=False)
    weights_ap = te.lower_ap(lhsT.opt(keep_dims), opt=False, for_matmul_weights=True)
    out_ap = te.lower_ap(out)
    tile_size = (_round_up(rhs.partition_size()), _round_up(out.partition_size()))
    tile_position = (lhsT.base_partition(), out.base_partition())
    return te.add_instruction(
        mybir.InstMatmult(
            name=te.bass.get_next_instruction_name(),
            replication_resolution=0,
            replication_shift_amnt=0,
            replication_num_rows=0,
            start_tensor_calc=start,
            stop_tensor_calc=stop,
            ins=[ifmap_ap, weights_ap],
            outs=[out_ap],
            perf_mode=None,
            is_transpose=None,
            ifmap_quant_offset=None,
            weights_quant_offset=None,
            bass_skip_group_check=False,
            tile_position=tile_position,
            tile_size=tile_size,
            ldweights=False,
        )
    )


def build_program(
    bpc=BPC,
    group_sizes=None,
    startup_split=False,
    ubufs=9,
    ebufs=8,
    wq_half=False,
    umult_gpsimd=False,
    kt_dve=False,
    v_kiouter=False,
    warmup=0,
    lag=4,
    last_lag=6,
    v_xbar=False,
    sc_ldw1=False,
    v_ldw1=False,
    out_bf16=True,
    proj_first=False,
    ctbufs=4,
    mmbufs=4,
    obufs=3,
    x_first=False,
):
    if group_sizes is None:
        group_sizes = (2, 2, 2, 2) if bpc == 8 else (bpc,)
    assert sum(group_sizes) == bpc
    MTOT = bpc * S

    nc = bacc.Bacc("TRN2", target_bir_lowering=False, debug=False, num_devices=1)
    xT_d = nc.dram_tensor("xT", [D, XW], BF16, kind="ExternalInput")
    wqT_d = nc.dram_tensor("wqT", [D, D], BF16, kind="ExternalInput")
    wkT_d = nc.dram_tensor("wkT", [D, D], BF16, kind="ExternalInput")
    wvT_d = nc.dram_tensor("wvT", [D, D], BF16, kind="ExternalInput")
    bq_d = nc.dram_tensor("bq2", [128, 2 * OT], F32, kind="ExternalInput")
    bv_d = nc.dram_tensor("bvb", [128, D], BF16, kind="ExternalInput")
    eb_d = nc.dram_tensor("expb", [128, H * IW2], BF16, kind="ExternalInput")
    on_d = nc.dram_tensor("onec", [128, H], BF16, kind="ExternalInput")
    out_d = nc.dram_tensor(
        "out", [MTOT, D], BF16 if out_bf16 else F32, kind="ExternalOutput"
    )

    with TileContext(nc) as tc:
        with (
            tc.tile_pool(name="const", bufs=1) as cp,
            tc.tile_pool(name="grp", bufs=2) as gp,
            tc.tile_pool(name="work", bufs=3) as wp,
            tc.tile_pool(name="ps", bufs=1, space="PSUM") as pp,
        ):
            wq_t = [
                cp.tile([128, D], BF16, name=f"wq{k}", tag=f"wq{k}") for k in range(KT)
            ]
            wk_t = [
                cp.tile([128, D], BF16, name=f"wk{k}", tag=f"wk{k}") for k in range(KT)
            ]
            wv_t = [
                cp.tile([128, D], BF16, name=f"wv{k}", tag=f"wv{k}") for k in range(KT)
            ]
            bqs = cp.tile([128, 2 * OT], F32, tag="bqs")
            bvb = cp.tile([128, D], BF16, tag="bvb")
            onec = cp.tile([128, H], BF16, tag="onec")
            ebf = cp.tile([128, H * IW2], BF16, tag="ebf")
            eb_t = [ebf[:, h * IW2 : (h + 1) * IW2] for h in range(H)]
            xf = [
                cp.tile([128, XW], BF16, name=f"xf{k}", tag=f"xf{k}") for k in range(KT)
            ]

            def load_small():
                nc.sync.dma_start(bqs[:], bq_d[:, :])
                nc.sync.dma_start(bvb[:], bv_d[:, :])
                nc.sync.dma_start(onec[:], on_d[:, :])

            def load_wq_x0():
                # wq + group-0's x slice interleaved: the first qt chain only
                # depends on these; o=0 weight columns land first so qt(0)
                # can start as soon as ~130KB are in.
                if startup_split:
                    for k in range(KT):
                        nc.sync.dma_start(
                            wq_t[k][:, :128], wqT_d[k * 128 : (k + 1) * 128, :128]
                        )
                        nc.sync.dma_start(
                            xf[k][:, :QW], xT_d[k * 128 : (k + 1) * 128, :QW]
                        )
                    for k in range(KT):
                        nc.sync.dma_start(
                            wq_t[k][:, 128:], wqT_d[k * 128 : (k + 1) * 128, 128:]
                        )
                elif wq_half:
                    for k in range(KT):
                        nc.sync.dma_start(
                            wq_t[k][:, :384], wqT_d[k * 128 : (k + 1) * 128, :384]
                        )
                        nc.sync.dma_start(
                            xf[k][:, :QW], xT_d[k * 128 : (k + 1) * 128, :QW]
                        )
                    for k in range(KT):
                        nc.sync.dma_start(
                            wq_t[k][:, 384:], wqT_d[k * 128 : (k + 1) * 128, 384:]
                        )
                elif x_first:
                    for k in range(KT):
                        nc.sync.dma_start(
                            xf[k][:, :QW], xT_d[k * 128 : (k + 1) * 128, :QW]
                        )
                        nc.sync.dma_start(wq_t[k][:], wqT_d[k * 128 : (k + 1) * 128, :])
                else:
                    for k in range(KT):
                        nc.sync.dma_start(wq_t[k][:], wqT_d[k * 128 : (k + 1) * 128, :])
                        nc.sync.dma_start(
                            xf[k][:, :QW], xT_d[k * 128 : (k + 1) * 128, :QW]
                        )

            def load_wk_xrest():
                # wk first: the kt chains stall on it; the bulk x transfer is
                # only needed by group 1's projections much later
                for k in range(KT):
                    nc.sync.dma_start(wk_t[k][:], wkT_d[k * 128 : (k + 1) * 128, :])
                for k in range(KT):
                    nc.sync.dma_start(
                        xf[k][:, QW:], xT_d[k * 128 : (k + 1) * 128, QW:]
                    )

            def load_wv():
                for k in range(KT):
                    nc.sync.dma_start(wv_t[k][:], wvT_d[k * 128 : (k + 1) * 128, :])

            def load_bias():
                nc.sync.dma_start(ebf[:], eb_d[:, :])

            def proj_pieces(g, GB, b0):
                """Emission thunks for group g's projections; last item is the
                ('ctx', dict) sentinel carrying the produced tiles."""
                m0 = b0 * S
                gw = GB * S + 2  # used stream width (<= QW tile width)
                ctx = {}

                def piece_load():
                    ctx["xt"] = [xf[k][:, m0 : m0 + gw] for k in range(KT)]
                    ctx["qt"] = [
                        gp.tile([128, QW], BF16, name=f"qt{o}", tag=f"qt{o}")
                        for o in range(OT)
                    ]
                    ctx["kt"] = [
                        gp.tile([128, QW], BF16, name=f"kt{o}", tag=f"kt{o}")
                        for o in range(OT)
                    ]
                    ctx["vt"] = {}

                yield piece_load

                def piece_qt(o):
                    xt, qt = ctx["xt"], ctx["qt"]
                    ps = pp.tile([128, 512], F32, name="pp", tag="mm512", bufs=mmbufs)
                    for ki in range(KT):
                        nc.tensor.matmul(
                            ps[:, :gw],
                            wq_t[ki][:, o * 128 : (o + 1) * 128],
                            xt[ki][:, :gw],
                            start=(ki == 0),
                            stop=(ki == KT - 1),
                        )
                    nc.scalar.activation(
                        qt[o][:, :gw],
                        ps[:, :gw],
                        ActFn.Identity,
                        bias=bqs[:, o : o + 1],
                    )

                def piece_kt(o):
                    xt, kt = ctx["xt"], ctx["kt"]
                    ps = pp.tile([128, 512], F32, name="pp", tag="mm512", bufs=mmbufs)
                    for ki in range(KT):
                        nc.tensor.matmul(
                            ps[:, :gw],
                            wk_t[ki][:, o * 128 : (o + 1) * 128],
                            xt[ki][:, :gw],
                            start=(ki == 0),
                            stop=(ki == KT - 1),
                        )
                    if kt_dve:
                        nc.vector.tensor_copy(kt[o][:, :gw], ps[:, :gw])
                    else:
                        nc.scalar.activation(kt[o][:, :gw], ps[:, :gw], ActFn.Identity)

                def piece_v(b, jt):
                    xt = ctx["xt"]
                    j0, jw = JT[jt]
                    v = gp.tile(
                        [128, H * 65], BF16, name=f"v{b}_{jt}", tag=f"v{b}_{jt}"
                    )
                    v3 = v[:jw, :].rearrange("p (h c) -> p h c", c=65)
                    chunks = [(0, 512, 0), (512, 256, 8)]
                    pss = [
                        pp.tile([128, 512], F32, name="pp", tag="mm512", bufs=mmbufs)
                        for _ in chunks
                    ]
                    if v_ldw1:
                        # ki outer with one explicit weight load per ki shared
                        # by both chunk matmuls
                        for ki in range(KT):
                            wap = xt[ki][:, b * S + j0 : b * S + j0 + jw]
                            nc.tensor.ldweights(wap)
                            for ps, (c0, cw, h0) in zip(pss, chunks):
                                matmul_noldw(
                                    nc.tensor,
                                    ps[:jw, :cw],
                                    wap,
                                    wv_t[ki][:, c0 : c0 + cw],
                                    start=(ki == 0),
                                    stop=(ki == KT - 1),
                                )
                    elif v_kiouter:
                        # ki outer: the two chunk matmuls per ki share the same
                        # stationary operand (the xt j-slice)
                        for ki in range(KT):
                            for ps, (c0, cw, h0) in zip(pss, chunks):
                                nc.tensor.matmul(
                                    ps[:jw, :cw],
                                    xt[ki][:, b * S + j0 : b * S + j0 + jw],
                                    wv_t[ki][:, c0 : c0 + cw],
                                    start=(ki == 0),
                                    stop=(ki == KT - 1),
                                )
                    else:
                        for ps, (c0, cw, h0) in zip(pss, chunks):
                            for ki in range(KT):
                                nc.tensor.matmul(
                                    ps[:jw, :cw],
                                    xt[ki][:, b * S + j0 : b * S + j0 + jw],
                                    wv_t[ki][:, c0 : c0 + cw],
                                    start=(ki == 0),
                                    stop=(ki == KT - 1),
                                )
                    for ps, (c0, cw, h0) in zip(pss, chunks):
                        nh = cw // 64
                        dst = v3[:, h0 : h0 + nh, 0:64]
                        src = ps[:jw, :cw].rearrange("p (h c) -> p h c", c=64)
                        bsl = bvb[:jw, c0 : c0 + cw].rearrange("p (h c) -> p h c", c=64)
                        nc.vector.tensor_tensor(dst, src, bsl, AluOp.add)
                    nc.vector.tensor_copy(
                        v3[:, :, 64:65],
                        onec[:jw, :].rearrange("p (h c) -> p h c", c=1),
                    )
                    ctx["vt"][b, jt] = v

                def piece_vT(o):
                    xt = ctx["xt"]
                    ps = pp.tile([128, 512], F32, name="pp", tag="mm512", bufs=mmbufs)
                    for ki in range(KT):
                        nc.tensor.matmul(
                            ps[:, :gw],
                            wv_t[ki][:, o * 128 : (o + 1) * 128],
                            xt[ki][:, :gw],
                            start=(ki == 0),
                            stop=(ki == KT - 1),
                        )
                    vT = gp.tile([128, QW + 64], BF16, name=f"vT{o}", tag=f"vT{o}")
                    nc.scalar.activation(
                        vT[:, :gw],
                        ps[:, :gw],
                        ActFn.Identity,
                        bias=bqs[:, OT + o : OT + o + 1],
                    )
                    ctx.setdefault("vT", {})[o] = vT

                def piece_vx(b, jt):
                    j0, jw = JT[jt]
                    jw2 = 128  # xbar transpose needs 128-divisible source cols
                    v = gp.tile(
                        [128, H * 65], BF16, name=f"v{b}_{jt}", tag=f"v{b}_{jt}"
                    )
                    v3p = v[:jw2, :].rearrange("p (h c) -> p h c", c=65)
                    for o in range(OT):
                        nc.sync.dma_start_transpose(
                            v3p[:, 2 * o : 2 * o + 2, 0:64],
                            ctx["vT"][o][:, b * S + j0 : b * S + j0 + jw2],
                        )
                    nc.vector.tensor_copy(
                        v[:jw, :].rearrange("p (h c) -> p h c", c=65)[:, :, 64:65],
                        onec[:jw, :].rearrange("p (h c) -> p h c", c=1),
                    )
                    ctx["vt"][b, jt] = v

                for o in range(OT):
                    yield (lambda o=o: piece_qt(o))
                for o in range(OT):
                    yield (lambda o=o: piece_kt(o))
                if v_xbar:
                    for o in range(OT):
                        yield (lambda o=o: piece_vT(o))
                    for b in range(GB):
                        for jt in range(2):
                            yield (lambda b=b, jt=jt: piece_vx(b, jt))
                else:
                    for b in range(GB):
                        for jt in range(2):
                            yield (lambda b=b, jt=jt: piece_v(b, jt))
                yield ("ctx", ctx)

            def att_pieces(GB, b0, ctx, lag=lag):
                """Emission thunks for a group's attention (lagged ctx stage).
                ctx is read lazily: tiles may be created mid-stream."""
                hgstate = {}

                def stage_scores_pair(b, hp):
                    qt, kt = ctx["qt"], ctx["kt"]
                    o = hp // 2
                    sts = [
                        pp.tile([128, 512], F32, name="st", tag="mm512", bufs=mmbufs)
                        for _ in range(2)
                    ]
                    # interleave the two heads so consecutive matmuls hit
                    # opposite PE row groups (partitions 0-63 vs 64-127) and
                    # overlap in the array
                    for jt, (j0, jw) in enumerate(JT):
                        if sc_ldw1:
                            # one full-array load covers both heads' row halves
                            nc.tensor.ldweights(
                                kt[o][0:128, b * S + j0 : b * S + j0 + jw]
                            )
                        for dh in (0, 1):
                            po = dh * 64
                            if sc_ldw1:
                                matmul_noldw(
                                    nc.tensor,
                                    sts[dh][:jw, jt * IW : (jt + 1) * IW],
                                    kt[o][po : po + 64, b * S + j0 : b * S + j0 + jw],
                                    qt[o][po : po + 64, b * S : b * S + IW],
                                    start=True,
                                    stop=True,
                                )
                            else:
                                nc.tensor.matmul(
                                    sts[dh][:jw, jt * IW : (jt + 1) * IW],
                                    kt[o][po : po + 64, b * S + j0 : b * S + j0 + jw],
                                    qt[o][po : po + 64, b * S : b * S + IW],
                                    start=True,
                                    stop=True,
                                )
                    out = []
                    for dh in (0, 1):
                        e = wp.tile([128, IW2], BF16, name="ee", tag="ee", bufs=ebufs)
                        nc.scalar.activation(e[:, :], sts[dh][:, :IW2], ActFn.Exp)
                        u = wp.tile([128, IW2], BF16, name="uu", tag="uu", bufs=ubufs)
                        ueng = nc.gpsimd if umult_gpsimd else nc.vector
                        ueng.tensor_tensor(
                            u[:, :], e[:, :], eb_t[hp + dh][:, :], AluOp.mult
                        )
                        out.append(u)
                    return out

                def stage_ctx(b, h, u, row0):
                    vt = ctx["vt"]
                    hg, hl = h // 6, h % 6
                    if hl == 0:
                        hgstate[b, hg] = [
                            pp.tile([128, 390], F32, name="ct", tag="ct", bufs=ctbufs)
                            for _ in range(2)
                        ]
                    cts = hgstate[b, hg]
                    for it, (i0, iw) in enumerate(IT):
                        for jt, (j0, jw) in enumerate(JT):
                            nc.tensor.matmul(
                                cts[it][:iw, hl * 65 : (hl + 1) * 65],
                                u[:jw, jt * IW + i0 : jt * IW + i0 + iw],
                                vt[b, jt][:jw, h * 65 : (h + 1) * 65],
                                start=(jt == 0),
                                stop=(jt == 1),
                            )
                    if hl == 5:
                        for it, (i0, iw) in enumerate(IT):
                            ct3 = cts[it][:iw, :].rearrange("p (h c) -> p h c", c=65)
                            rt = wp.tile([128, 6], F32, name="rt", tag="rt")
                            rt3 = rt[:iw, :].rearrange("p (h c) -> p h c", c=1)
                            nc.vector.reciprocal(rt3, ct3[:, :, 64:65])
                            num = ct3[:, :, 0:64]
                            _, rb3 = bass_broadcast(num, rt3)
                            ob = wp.tile(
                                [128, 384], BF16 if out_bf16 else F32, name="ob", tag="ob", bufs=obufs
                            )
                            nc.vector.tensor_tensor(
                                ob[:iw, :].rearrange("p (h c) -> p h c", c=64),
                                num,
                                rb3,
                                AluOp.mult,
                            )
                            nc.sync.dma_start(
                                out_d[
                                    row0 + i0 : row0 + i0 + iw,
                                    hg * 384 : (hg + 1) * 384,
                                ],
                                ob[:iw, :],
                            )

                pend = deque()
                for b in range(GB):
                    for hp in range(0, H, 2):

                        def piece(b=b, hp=hp):
                            us01 = stage_scores_pair(b, hp)
                            for dh in (0, 1):
                                pend.append((b, hp + dh, us01[dh], (b0 + b) * S))
                            while len(pend) > lag:
                                stage_ctx(*pend.popleft())

                        yield piece

                def flush():
                    while pend:
                        stage_ctx(*pend.popleft())

                yield flush

            def run_proj(gen):
                pieces = []
                ctx = None
                for item in gen:
                    if isinstance(item, tuple) and item[0] == "ctx":
                        ctx = item[1]
                    else:
                        pieces.append(item)
                return pieces, ctx

            b0s = []
            acc = 0
            for GB in group_sizes:
                b0s.append(acc)
                acc += GB

            g0_pieces, prev_ctx = run_proj(proj_pieces(0, group_sizes[0], b0s[0]))
            load_small()
            if warmup:
                # dummy matmuls on a tiny early tile: keep the PE busy during
                # the input DMA lead-in so HAM ramps the clock before real work
                dum = cp.tile([128, 512], BF16, tag="dum")
                nc.sync.dma_start(dum[:], wqT_d[0:128, 0:512])
                for w in range(warmup):
                    psw = pp.tile([128, 512], F32, name="pp", tag="mm512", bufs=mmbufs)
                    nc.tensor.matmul(
                        psw[:, :512],
                        dum[:, 0:128],
                        dum[:, :512],
                        start=True,
                        stop=True,
                    )
            load_wq_x0()
            g0_pieces[0]()  # tile setup only
            pq0 = g0_pieces[1 : 1 + OT]
            pk0 = g0_pieces[1 + OT : 1 + 2 * OT]
            pv0 = g0_pieces[1 + 2 * OT :]
            pq0[0]()
            load_wk_xrest()
            for p in pq0[1:]:
                p()
            load_wv()
            for p in pk0:
                p()
            load_bias()
            for p in pv0:
                p()

            def interleave(astream, pstream):
                ratio = max(1, len(astream) // max(1, len(pstream)))
                out = []
                ai = pi = 0
                while ai < len(astream) or pi < len(pstream):
                    if proj_first and pi < len(pstream):
                        out.append(pstream[pi])
                        pi += 1
                    for _ in range(ratio):
                        if ai < len(astream):
                            out.append(astream[ai])
                            ai += 1
                    if not proj_first and pi < len(pstream):
                        out.append(pstream[pi])
                        pi += 1
                return out

            ng = len(group_sizes)
            for g in range(1, ng - 1):
                pieces, g_ctx = run_proj(proj_pieces(g, group_sizes[g], b0s[g]))
                for p in interleave(
                    list(att_pieces(group_sizes[g - 1], b0s[g - 1], prev_ctx)), pieces
                ):
                    p()
                prev_ctx = g_ctx

            if ng == 1:
                for p in att_pieces(group_sizes[0], b0s[0], prev_ctx):
                    p()
            else:
                # final window: att(gl-1) interleaved with the last group's
                # load/qt/kt0-2 pieces; kt3-5 + V pieces are deferred into the
                # last group's own attention stream as just-in-time PE filler.
                gl = ng - 1
                pieces, gl_ctx = run_proj(proj_pieces(gl, group_sizes[gl], b0s[gl]))
                pload = pieces[0]
                pqt = pieces[1 : 1 + OT]
                pkt = pieces[1 + OT : 1 + 2 * OT]
                window = [pload] + pqt + pkt[:3]
                for p in interleave(
                    list(att_pieces(group_sizes[gl - 1], b0s[gl - 1], prev_ctx)),
                    window,
                ):
                    p()
                if v_xbar:
                    pvT = pieces[1 + 2 * OT : 1 + 3 * OT]
                    pvx = pieces[1 + 3 * OT :]
                    apieces = list(
                        att_pieces(group_sizes[gl], b0s[gl], gl_ctx, lag=8)
                    )
                    aflush = apieces[-1]
                    A = apieces[:-1]
                    sched = {
                        0: [pvT[0], pvT[1]],
                        1: [pvT[2], pvT[3]],
                        2: [pvT[4], pvT[5]],
                        3: [pkt[3]] + pvx[0:2],
                        4: [pkt[4]],
                        5: [pkt[5]],
                        6: pvx[2:],
                    }
                    out_stream = []
                    for idx, a in enumerate(A):
                        out_stream.extend(sched.get(idx, []))
                        out_stream.append(a)
                    out_stream.append(aflush)
                    for p in out_stream:
                        p()
                else:
                    pv = deque(pieces[1 + 2 * OT :])
                    apieces = list(
                        att_pieces(group_sizes[gl], b0s[gl], gl_ctx, lag=last_lag)
                    )
                    aflush = apieces[-1]
                    A = apieces[:-1]
                    out_stream = []
                    if last_lag >= 8:
                        pv_sched = {0: 2, 8: 1, 9: 1}
                    elif last_lag >= 6:
                        # deeper lag delays the second batch's ctx pops, so its
                        # V pieces can fill the otherwise-empty late pairs
                        pv_sched = {0: 2, 7: 1, 8: 1}
                    else:
                        pv_sched = {0: 2, 5: 1, 6: 1}
                    for idx, a in enumerate(A):
                        if 3 <= idx < OT:
                            out_stream.append(pkt[idx])  # kt[idx] just before its pair
                        out_stream.append(a)
                        for _ in range(pv_sched.get(idx, 0)):
                            if pv:
                                out_stream.append(pv.popleft())
                    while pv:
                        out_stream.append(pv.popleft())
                    out_stream.append(aflush)
                    for p in out_stream:
                        p()

    nc.compile()
    return nc


def prep_host_inputs(inputs, bpc=BPC, cores=NCORES):
    """Shared (per-core-identical) tensors + per-core xT shards."""
    hs = np.ascontiguousarray(np.asarray(inputs["hidden_states"], dtype=np.float32))
    wq = np.asarray(inputs["wq"], np.float32)
    bq = np.asarray(inputs["bq"], np.float32)
    wk = np.asarray(inputs["wk"], np.float32)
    wv = np.asarray(inputs["wv"], np.float32)
    bv = np.asarray(inputs["bv"], np.float32)
    bias_table = np.asarray(inputs["bias_table"], np.float32)
    rel_index = np.asarray(inputs["rel_index"])

    bf = ml_dtypes.bfloat16
    scale = np.float32(1.0 / np.sqrt(HD))
    common = {
        "wqT": np.ascontiguousarray(wq.T * scale).astype(bf),
        "wkT": np.ascontiguousarray(wk.T).astype(bf),
        "wvT": np.ascontiguousarray(wv.T).astype(bf),
        "bq2": np.ascontiguousarray(
            np.concatenate(
                [(bq * scale).reshape(OT, 128).T, bv.reshape(OT, 128).T], axis=1
            )
        ),
        "bvb": np.ascontiguousarray(np.broadcast_to(bv, (128, D))).astype(bf),
        "onec": np.ones((128, H), bf),
    }
    rb = bias_table[rel_index]  # [i, j, H]
    bjiT = rb.transpose(2, 1, 0)  # [h, j, i]
    eb = np.zeros((H, 128, IW2), np.float32)
    for jt, (j0, jw) in enumerate(JT):
        eb[:, :jw, jt * IW : jt * IW + S] = np.exp(bjiT[:, j0 : j0 + jw, :])
    # [h, p, c] -> [p, h*IW2 + c]
    common["expb"] = np.ascontiguousarray(eb.transpose(1, 0, 2).reshape(128, H * IW2)).astype(bf)

    in_maps = []
    for c in range(cores):
        xc = hs[c * bpc : (c + 1) * bpc].reshape(bpc * S, D)
        xT = np.zeros((D, XW), np.float32)
        xT[:, : bpc * S] = xc.T
        in_maps.append({"xT": xT.astype(bf), **common})
    return in_maps


_prog_cache = {}


def get_program(bpc=BPC, group_sizes=None, **kw):
    key = (bpc, group_sizes, tuple(sorted(kw.items())))
    if key not in _prog_cache:
        _prog_cache[key] = build_program(bpc, group_sizes, **kw)
    return _prog_cache[key]


def kernel(**inputs):
    nc = get_program()
    in_maps = prep_host_inputs(inputs)
    res = run_bass_kernel_spmd(nc, in_maps, list(range(NCORES)))
    out = np.concatenate(
        [np.asarray(res.results[c]["out"], dtype=np.float32) for c in range(NCORES)],
        axis=0,
    )
    return out.reshape(B, S, D)
